# revision 37
# baseline (speedup 1.0000x reference)
"""Trainium2 Bass kernel for Longformer self-attention (B=2, S=4096, D=768, H=12, HD=64, W=256, G=32).

Sharding: 8 cores = 2 batches x 4 head-groups (3 heads each). Each core computes its
batch's projections restricted to its 192 output channels, runs banded + global
attention for its 3 heads, and returns an unnormalized transposed output
([3, 65, S]: rows 0-63 = head-dim, row 64 = softmax denominator z) plus the raw
global-query output [3, G, 65]; the host divides by z, transposes, and assembles.

Key design (v4):
  - All score matmuls run at K=128 full-array rate via ZERO-PADDED per-head
    query tiles (qZ[h] holds head h's 64 q-channels on the partition range of
    its k data, zeros elsewhere; the packed k tiles' cross-head terms are
    killed by the zeros). K=64 matmuls with changing weights cannot pipeline
    on the PE (measured fill+drain serialization, ~2x cost) -- avoided.
  - hidden_states pre-transposed on host -> contiguous DMA loads; startup DMAs
    sliced across the three DMA-issuing engines (sync/scalar/gpsimd).
  - 5 projection chains of full 128 output rows (q01, k01, kg01, [q2|kg2],
    [k2|qg2]).
  - band score blocks interleaved between projection/PV chains (same PE mode)
    so PSUM-slot waits on the softmax exp hide behind useful matmuls.
  - band mask applied as one strided bf16 multiply on the exp'd probs.
  - global-key PV contribution stays K=128 via zero-padded operands
    (exp_sg rows 96:128 = 0, per-head global-v at partitions 32h:32h+32).
  - v/vg bias via pre-broadcast bf16 add fused into the PSUM->SBUF copy.
Matmul inputs bf16, fp32 PSUM/softmax.
"""
from collections import deque

import numpy as np
import ml_dtypes

import concourse.bass as bass
import concourse.mybir as mybir
import concourse.tile as tile
from concourse import bacc
from concourse.bass_utils import run_bass_kernel_spmd

B, S, D, H, HD = 2, 4096, 768, 12, 64
W = 256
G = 32
SCALE = 1.0 / np.float32(np.sqrt(HD))
KB = 128
NKB = S // KB     # 32
QSB = 512
NQSB = S // QSB   # 8
NKT = D // 128    # 6
NNT = S // 512    # 8

BF = mybir.dt.bfloat16
F32 = mybir.dt.float32
AF = mybir.ActivationFunctionType
bf16 = ml_dtypes.bfloat16

_cache = {}
PUMP_INTERLEAVE = True


def _span(kb):
    k0 = KB * kb
    qlo, qhi = max(0, k0 - 2 * KB), min(S, k0 + 3 * KB)
    return qlo, qhi, qlo - (k0 - 2 * KB), qhi - (k0 - 2 * KB)


def _build():
    nc = bacc.Bacc(None, target_bir_lowering=False)

    hsT_d = nc.declare_dram_parameter("hsT", [128, NNT, NKT, 512], BF, isOutput=False)
    w5_d = nc.declare_dram_parameter("w5", [128, 5, NKT, 128], BF, isOutput=False)
    wqg_d = nc.declare_dram_parameter("wqg01", [128, NKT, 128], BF, isOutput=False)
    wvvg_d = nc.declare_dram_parameter("wvvg", [128, NKT, 384], BF, isOutput=False)
    bvvg_d = nc.declare_dram_parameter("bvvg", [1, 384], BF, isOutput=False)
    bias_d = nc.declare_dram_parameter("bias_t", [128, 8], F32, isOutput=False)
    masks_d = nc.declare_dram_parameter("masks", [128, 2, 128], BF, isOutput=False)
    id96_d = nc.declare_dram_parameter("id96", [96, 96], BF, isOutput=False)
    out_d = nc.declare_dram_parameter("out", [3, 65, S], F32, isOutput=True)
    outg_d = nc.declare_dram_parameter("outg", [3, G, 65], F32, isOutput=True)

    with tile.TileContext(nc) as tc:
        with tc.tile_pool(name="persist", bufs=1) as pp:
            masks_t = pp.tile([128, 2, 128], BF)
            ones_t = pp.tile([1, 128], BF)

            # per-head zero-padded q; head h's live rows match its k tile rows
            qZ = [pp.tile([128, S], BF, name=f"qZ{i}") for i in range(3)]
            kT01 = pp.tile([128, S], BF)   # k: h0 rows 0:64, h1 rows 64:128
            kT2 = pp.tile([128, S], BF)    # k: h2 rows 0:64, rows 64:128 zero
            v_nat = pp.tile([128, NKB, 3, 65], BF)
            # exp_sg: rows 32h..32h+31 = head h's exp'd global-key scores;
            # rows 96:128 stay zero so K=128 matmuls vs vGp are exact.
            exp_sg = pp.tile([128, S], BF)

            with tc.tile_pool(name="ac", bufs=1) as ac:
                kgT01 = ac.tile([128, S], BF)
                kgT2 = ac.tile([128, S], BF)   # h2 rows 64:128, rows 0:64 zero
                qgZ = [ac.tile([128, G], BF, name=f"qgZ{i}") for i in range(3)]
                vg_nat = ac.tile([128, NKB, 3, 65], BF)
                vGp = ac.tile([128, 3, 65], BF)   # head h global-v at rows 32h:32h+32
                probs_g = ac.tile([96, S], BF)
                pb_gT = ac.tile([128, NKB, 96], BF)
                id96_t = ac.tile([96, 96], BF)
                bvvg_b = ac.tile([128, 384], BF)  # bias broadcast over tokens

                with (
                    tc.tile_pool(name="aw", bufs=1) as aw,
                    tc.tile_pool(name="hst", bufs=2) as hstp,
                    tc.tile_pool(name="apsum", bufs=2, space="PSUM") as apsum,
                    tc.tile_pool(name="spsum", bufs=2, space="PSUM") as spsum,
                    tc.tile_pool(name="opsum", bufs=2, space="PSUM") as opsum,
                    tc.tile_pool(name="pbt", bufs=42) as pbtp,
                    tc.tile_pool(name="osb", bufs=2) as osbp,
                ):
                    w5_t = aw.tile([128, 5, NKT, 128], BF)
                    wqg_t = aw.tile([128, NKT, 128], BF)
                    wvvg_t = aw.tile([128, NKT, 384], BF)
                    bvvg_t = aw.tile([1, 384], BF)
                    bias_t = aw.tile([128, 8], F32)
                    # issue startup DMAs sliced across the three DMA-issuing
                    # engines so the critical-path transfers run in parallel
                    hst0 = hstp.tile([128, NKT, 512], BF)
                    nc.sync.dma_start(w5_t[:, 0], w5_d[:, 0])
                    nc.scalar.dma_start(bias_t[:], bias_d[:])
                    nc.gpsimd.dma_start(hst0[:, 2:4, :], hsT_d[:, 0, 2:4, :])
                    nc.scalar.dma_start(hst0[:, 0:2, :], hsT_d[:, 0, 0:2, :])
                    nc.sync.dma_start(hst0[:, 4:6, :], hsT_d[:, 0, 4:6, :])
                    nc.scalar.dma_start(w5_t[:, 1], w5_d[:, 1])
                    nc.gpsimd.dma_start(w5_t[:, 2], w5_d[:, 2])
                    nc.sync.dma_start(w5_t[:, 3], w5_d[:, 3])
                    nc.scalar.dma_start(w5_t[:, 4], w5_d[:, 4])
                    nc.gpsimd.dma_start(wvvg_t[:], wvvg_d[:])
                    nc.sync.dma_start(bvvg_t[:], bvvg_d[:])
                    nc.scalar.dma_start(masks_t[:], masks_d[:])
                    nc.gpsimd.dma_start(wqg_t[:], wqg_d[:])
                    nc.gpsimd.dma_start(id96_t[:], id96_d[:])
                    # zero-fill the dead halves of the padded tiles + ones cols
                    nc.gpsimd.memset(ones_t[:], 1.0)
                    nc.gpsimd.memset(qZ[0][64:128, :], 0.0)
                    nc.gpsimd.memset(qZ[1][0:64, :], 0.0)
                    nc.gpsimd.memset(qZ[2][64:128, :], 0.0)
                    nc.gpsimd.memset(kT2[64:128, :], 0.0)
                    nc.gpsimd.memset(kgT2[0:64, :], 0.0)
                    nc.gpsimd.memset(qgZ[0][64:128, :], 0.0)
                    nc.gpsimd.memset(qgZ[1][0:64, :], 0.0)
                    nc.gpsimd.memset(qgZ[2][0:64, :], 0.0)
                    nc.gpsimd.memset(exp_sg[96:128, :], 0.0)
                    nc.gpsimd.memset(v_nat[:, :, :, 64:65], 1.0)
                    nc.gpsimd.memset(vg_nat[:, :, :, 64:65], 1.0)
                    nc.gpsimd.memset(vGp[:], 0.0)
                    for h in range(3):
                        nc.gpsimd.memset(vGp[32 * h:32 * h + 32, h, 64:65], 1.0)

                    pbt = {}
                    band_q = deque()
                    state = {"kb_done": 0, "qs_done": 0, "kb_next": 0}

                    def ktile(h):
                        return kT01 if h < 2 else kT2

                    def mm_score(t, h, kb, a, b2):
                        k0 = KB * kb
                        qlo, qhi, llo, lhi = _span(kb)
                        nc.tensor.matmul(
                            t[:, a:b2],
                            ktile(h)[:, k0:k0 + KB],
                            qZ[h][:, qlo + (a - llo):qlo + (a - llo) + (b2 - a)])

                    def exp_mask(ps, kb, h):
                        qlo, qhi, llo, lhi = _span(kb)
                        t_ = pbtp.tile([128, 640], BF, tag="pb")
                        nc.scalar.activation(t_[:, llo:lhi], ps[:, llo:lhi], AF.Exp)
                        if llo == 0:
                            nc.gpsimd.tensor_mul(t_[:, 0:128], t_[:, 0:128],
                                                 masks_t[:, 0, :])
                        if lhi == 640:
                            nc.gpsimd.tensor_mul(t_[:, 512:640], t_[:, 512:640],
                                                 masks_t[:, 1, :])
                        pbt[(kb, h)] = t_

                    def queue_kb(kb):
                        qlo, qhi, llo, lhi = _span(kb)
                        pieces = [(a, b) for (a, b) in
                                  [(llo, min(lhi, 512)), (max(llo, 512), lhi)] if a < b]

                        def part1():
                            t0 = spsum.tile([128, 640], F32, tag="sc", name=f"s{kb}_0")
                            t1 = spsum.tile([128, 640], F32, tag="sc", name=f"s{kb}_1")
                            for (a, b2) in pieces:
                                mm_score(t0, 0, kb, a, b2)
                                mm_score(t1, 1, kb, a, b2)
                            exp_mask(t0, kb, 0)
                            exp_mask(t1, kb, 1)

                        def part2():
                            t2 = spsum.tile([128, 640], F32, tag="sc", name=f"s{kb}_2")
                            for (a, b2) in pieces:
                                mm_score(t2, 2, kb, a, b2)
                            exp_mask(t2, kb, 2)
                            state["kb_done"] += 1

                        band_q.append(part1)
                        band_q.append(part2)

                    def pump(n=1):
                        if PUMP_INTERLEAVE:
                            for _ in range(n):
                                if band_q:
                                    band_q.popleft()()

                    def pump_all():
                        while band_q:
                            band_q.popleft()()

                    def do_pv(qs):
                        q0 = QSB * qs
                        kbs = list(range(max(0, 4 * qs - 2), min(NKB, 4 * qs + 6)))
                        for h in range(3):
                            po = opsum.tile([96, 512], F32, tag="po")
                            nc.tensor.matmul(po[0:65, :], vGp[:, h, :],
                                             exp_sg[:, q0:q0 + 512],
                                             start=True, stop=False)
                            for i, kb in enumerate(kbs):
                                k0 = KB * kb
                                qlo, qhi, llo, lhi = _span(kb)
                                a, b2 = max(qlo, q0), min(qhi, q0 + QSB)
                                la = a - (k0 - 2 * KB)
                                nc.tensor.matmul(po[0:65, a - q0:b2 - q0],
                                                 v_nat[:, kb, h, :],
                                                 pbt[(kb, h)][:, la:la + (b2 - a)],
                                                 start=False, stop=(i == len(kbs) - 1))
                            ob = osbp.tile([65, 512], F32, tag="ob")
                            nc.vector.tensor_copy(ob[:], po[0:65, :])
                            nc.sync.dma_start(out_d[h, :, q0:q0 + 512], ob[:])
                            pump()

                    def pump_pv(nt):
                        while state["qs_done"] < NQSB \
                                and 4 * state["qs_done"] + 5 <= state["kb_done"] - 1 \
                                and state["qs_done"] <= nt - 1:
                            do_pv(state["qs_done"])
                            state["qs_done"] += 1

                    def do_tpose(blk):
                        pst = apsum.tile([128, 96], BF, tag="pp", name=f"tp{blk}")
                        nc.tensor.transpose(pst[:], probs_g[:, 128 * blk:128 * blk + 128],
                                            id96_t[:])
                        nc.vector.tensor_copy(pb_gT[:, blk, :], pst[:])

                    for nt in range(NNT):
                        c0 = 512 * nt
                        if nt == 0:
                            hst = hst0
                        else:
                            hst = hstp.tile([128, NKT, 512], BF)
                            nc.sync.dma_start(hst[:], hsT_d[:, nt, :, :])
                        # queue band blocks whose q/k spans are fully written
                        # (chunks <= nt-1); the rest after this chunk's m-chains
                        while state["kb_next"] <= min(4 * nt - 3, NKB - 1):
                            queue_kb(state["kb_next"])
                            state["kb_next"] += 1
                        for c in range(5):
                            ps = apsum.tile([128, 512], F32, tag="pp")
                            for kt in range(NKT):
                                nc.tensor.matmul(ps[:], w5_t[:, c, kt, :],
                                                 hst[:, kt, :],
                                                 start=(kt == 0), stop=(kt == NKT - 1))
                            if c == 0:
                                nc.vector.tensor_scalar_add(
                                    qZ[0][0:64, c0:c0 + 512], ps[0:64, :],
                                    bias_t[0:64, 0:1])
                                nc.vector.tensor_scalar_add(
                                    qZ[1][64:128, c0:c0 + 512], ps[64:128, :],
                                    bias_t[64:128, 0:1])
                            elif c == 1:
                                nc.vector.tensor_scalar_add(
                                    kT01[:, c0:c0 + 512], ps[:], bias_t[:, 1:2])
                            elif c == 2:
                                nc.vector.tensor_scalar_add(
                                    kgT01[:, c0:c0 + 512], ps[:], bias_t[:, 2:3])
                            elif c == 3:
                                nc.vector.tensor_scalar_add(
                                    qZ[2][0:64, c0:c0 + 512], ps[0:64, :],
                                    bias_t[0:64, 3:4])
                                nc.vector.tensor_scalar_add(
                                    kgT2[64:128, c0:c0 + 512], ps[64:128, :],
                                    bias_t[64:128, 3:4])
                            else:
                                nc.vector.tensor_scalar_add(
                                    kT2[0:64, c0:c0 + 512], ps[0:64, :],
                                    bias_t[0:64, 4:5])
                                if nt == 0:
                                    nc.vector.tensor_scalar_add(
                                        qgZ[2][64:128, :], ps[64:128, 0:G],
                                        bias_t[64:128, 4:5])
                            pump()
                        while state["kb_next"] <= min(4 * nt + 1, NKB - 1):
                            queue_kb(state["kb_next"])
                            state["kb_next"] += 1
                        if nt == 0:
                            psq = apsum.tile([128, 512], F32, tag="pp")
                            for kt in range(NKT):
                                nc.tensor.matmul(psq[:, 0:G], wqg_t[:, kt, :],
                                                 hst[:, kt, 0:G],
                                                 start=(kt == 0), stop=(kt == NKT - 1))
                            nc.vector.tensor_scalar_add(qgZ[0][0:64, :], psq[0:64, 0:G],
                                                        bias_t[0:64, 5:6])
                            nc.vector.tensor_scalar_add(qgZ[1][64:128, :],
                                                        psq[64:128, 0:G],
                                                        bias_t[64:128, 5:6])
                            # broadcast bvvg to all partitions via ones-matmul
                            psb = apsum.tile([128, 512], F32, tag="pp")
                            nc.tensor.matmul(psb[:, 0:384], ones_t[:, 0:128], bvvg_t[:],
                                             start=True, stop=True)
                            nc.vector.tensor_copy(bvvg_b[:], psb[:, 0:384])
                        for s4 in range(4):
                            sb = 4 * nt + s4
                            psv = apsum.tile([128, 512], F32, tag="pp")
                            for kt in range(NKT):
                                nc.tensor.matmul(psv[:, 0:384],
                                                 hst[:, kt, 128 * s4:128 * s4 + 128],
                                                 wvvg_t[:, kt, :],
                                                 start=(kt == 0), stop=(kt == NKT - 1))
                            nc.vector.tensor_add(
                                v_nat[:, sb, :, 0:64],
                                psv[:, 0:192].rearrange("p (h e) -> p h e", h=3),
                                bvvg_b[:, 0:192].rearrange("p (h e) -> p h e", h=3))
                            nc.vector.tensor_add(
                                vg_nat[:, sb, :, 0:64],
                                psv[:, 192:384].rearrange("p (h e) -> p h e", h=3),
                                bvvg_b[:, 192:384].rearrange("p (h e) -> p h e", h=3))
                            pump()
                        if nt == 0:
                            # per-head global v/ones at partitions 32h:32h+32
                            nc.vector.tensor_copy(vGp[0:32, 0, 0:64],
                                                  v_nat[0:32, 0, 0, 0:64])
                            nc.sync.dma_start(vGp[32:64, 1, 0:64],
                                              v_nat[0:32, 0, 1, 0:64])
                            nc.sync.dma_start(vGp[64:96, 2, 0:64],
                                              v_nat[0:32, 0, 2, 0:64])
                        if nt >= 1:
                            for blk in range(4 * (nt - 1), 4 * nt):
                                do_tpose(blk)
                                pump()
                        pump_pv(nt)
                        pump(2)
                        # ---- (128,32)-col-tiled scores: sg + global-query ----
                        pssg = opsum.tile([96, 512], F32, tag="po", name=f"sg{nt}")
                        for h in range(3):
                            nc.tensor.matmul(pssg[32 * h:32 * h + 32, :],
                                             ktile(h)[:, 0:G],
                                             qZ[h][:, c0:c0 + 512])
                        nc.scalar.activation(exp_sg[0:96, c0:c0 + 512], pssg[:], AF.Exp)
                        psgc = opsum.tile([96, 512], F32, tag="po", name=f"gc{nt}")
                        for h in range(3):
                            nc.tensor.matmul(psgc[32 * h:32 * h + 32, :],
                                             qgZ[h][:],
                                             (kgT01 if h < 2 else kgT2)[:, c0:c0 + 512])
                        nc.scalar.activation(probs_g[:, c0:c0 + 512], psgc[:], AF.Exp)
                        pump(2)

                    # tail: drain band queue, pv(6); the global-query PV chain
                    # runs before pv(7) so its copies/DMA overlap pv(7)'s matmuls
                    pump_all()
                    while state["qs_done"] <= 6:
                        do_pv(state["qs_done"])
                        state["qs_done"] += 1
                    for blk in range(4 * (NNT - 1), NKB):
                        do_tpose(blk)
                    while state["kb_next"] < NKB:
                        queue_kb(state["kb_next"])
                        state["kb_next"] += 1
                    pump_all()
                    do_pv(7)
                    go = apsum.tile([128, 512], F32, tag="pp", name="go")
                    og_sb = osbp.tile([96, 65], F32, tag="og")
                    for t in range(NKB):
                        for h in range(3):
                            nc.tensor.matmul(go[32 * h:32 * h + 32, 0:65],
                                             pb_gT[:, t, 32 * h:32 * h + 32],
                                             vg_nat[:, t, h, :],
                                             start=(t == 0), stop=(t == NKB - 1))
                    nc.vector.tensor_copy(og_sb[:], go[0:96, 0:65])
                    nc.sync.dma_start(outg_d[0], og_sb[0:32, :])
                    nc.scalar.dma_start(outg_d[1], og_sb[32:64, :])
                    nc.gpsimd.dma_start(outg_d[2], og_sb[64:96, :])

    nc.compile()
    return nc


def _prep_inputs(inputs):
    hs = np.asarray(inputs["hidden_states"], dtype=np.float32)
    j = np.arange(KB)[None, :]
    p = np.arange(KB)[:, None]
    masks = np.stack([(j >= p), (j <= p)], axis=1).astype(bf16)  # [128, 2, 128]
    id96 = np.eye(96, dtype=bf16)

    def wtiles(w):
        n = w.shape[1]
        return np.ascontiguousarray(w.reshape(NKT, 128, n).transpose(1, 0, 2)).astype(bf16)

    maps = []
    for c in range(8):
        b, hg = c // 4, c % 4
        cols = slice(192 * hg, 192 * hg + 192)
        Wq = np.asarray(inputs["Wq"], np.float32)[:, cols] * SCALE
        bq = np.asarray(inputs["bq"], np.float32)[cols] * SCALE
        Wqg = np.asarray(inputs["Wqg"], np.float32)[:, cols] * SCALE
        bqg = np.asarray(inputs["bqg"], np.float32)[cols] * SCALE
        Wk = np.asarray(inputs["Wk"], np.float32)[:, cols]
        bk = np.asarray(inputs["bk"], np.float32)[cols]
        Wkg = np.asarray(inputs["Wkg"], np.float32)[:, cols]
        bkg = np.asarray(inputs["bkg"], np.float32)[cols]
        Wv = np.asarray(inputs["Wv"], np.float32)[:, cols]
        bv = np.asarray(inputs["bv"], np.float32)[cols]
        Wvg = np.asarray(inputs["Wvg"], np.float32)[:, cols]
        bvg = np.asarray(inputs["bvg"], np.float32)[cols]

        # 5 chains: q01, k01, kg01, [q2|kg2], [k2|qg2]
        w5 = np.concatenate([
            Wq[:, 0:128],
            Wk[:, 0:128],
            Wkg[:, 0:128],
            np.concatenate([Wq[:, 128:192], Wkg[:, 128:192]], axis=1),
            np.concatenate([Wk[:, 128:192], Wqg[:, 128:192]], axis=1),
        ], axis=1)  # [768, 640]
        w5_t = np.ascontiguousarray(
            wtiles(w5).reshape(128, NKT, 5, 128).transpose(0, 2, 1, 3))

        bias_t = np.zeros((128, 8), np.float32)
        bias_t[:, 0] = bq[0:128]
        bias_t[:, 1] = bk[0:128]
        bias_t[:, 2] = bkg[0:128]
        bias_t[0:64, 3], bias_t[64:128, 3] = bq[128:192], bkg[128:192]
        bias_t[0:64, 4], bias_t[64:128, 4] = bk[128:192], bqg[128:192]
        bias_t[0:64, 5], bias_t[64:128, 5] = bqg[0:64], bqg[64:128]

        hsT = np.ascontiguousarray(
            hs[b].T.reshape(NKT, 128, NNT, 512).transpose(1, 2, 0, 3)).astype(bf16)

        maps.append({
            "hsT": hsT,
            "w5": w5_t,
            "wqg01": wtiles(Wqg[:, 0:128]),
            "wvvg": wtiles(np.concatenate([Wv, Wvg], axis=1)),
            "bvvg": np.concatenate([bv, bvg])[None, :].astype(bf16),
            "bias_t": bias_t,
            "masks": masks,
            "id96": id96,
        })
    return maps


def kernel(**inputs):
    g = int(np.asarray(inputs["num_global"]))
    assert g == G, f"kernel compiled for num_global=32, got {g}"
    if "nc" not in _cache:
        _cache["nc"] = _build()
    nc = _cache["nc"]
    in_maps = _prep_inputs(inputs)
    res = run_bass_kernel_spmd(nc, in_maps, list(range(8)))
    return assemble(res.results)


def assemble(results):
    out = np.zeros((B, S, D), np.float32)
    for c in range(8):
        b, hg = c // 4, c % 4
        o = results[c]["out"]          # [3, 65, S]
        og = results[c]["outg"]        # [3, G, 65]
        for h in range(3):
            col = 192 * hg + 64 * h
            out[b, :, col:col + 64] = (o[h, 0:64] / o[h, 64]).T
            out[b, 0:G, col:col + 64] = og[h, :, 0:64] / og[h, :, 64:65]
    return out


# revision 38
# speedup vs baseline: 1.0482x; 1.0482x over previous
"""Trainium2 Bass kernel for Longformer self-attention (B=2, S=4096, D=768, H=12, HD=64, W=256, G=32).

Sharding: 8 cores = 2 batches x 4 head-groups (3 heads each). Each core computes its
batch's projections restricted to its 192 output channels, runs banded + global
attention for its 3 heads, and returns an unnormalized transposed output
([3, 65, S]: rows 0-63 = head-dim, row 64 = softmax denominator z) plus the raw
global-query output [3, G, 65]; the host divides by z, transposes, and assembles.

Key design (v4):
  - All score matmuls run at K=128 full-array rate via ZERO-PADDED per-head
    query tiles (qZ[h] holds head h's 64 q-channels on the partition range of
    its k data, zeros elsewhere; the packed k tiles' cross-head terms are
    killed by the zeros). K=64 matmuls with changing weights cannot pipeline
    on the PE (measured fill+drain serialization, ~2x cost) -- avoided.
  - hidden_states pre-transposed on host -> contiguous DMA loads; startup DMAs
    sliced across the three DMA-issuing engines (sync/scalar/gpsimd).
  - 5 projection chains of full 128 output rows (q01, k01, kg01, [q2|kg2],
    [k2|qg2]).
  - band score blocks interleaved between projection/PV chains (same PE mode)
    so PSUM-slot waits on the softmax exp hide behind useful matmuls.
  - band mask applied as one strided bf16 multiply on the exp'd probs.
  - global-key PV contribution stays K=128 via zero-padded operands
    (exp_sg rows 96:128 = 0, per-head global-v at partitions 32h:32h+32).
  - v/vg bias via pre-broadcast bf16 add fused into the PSUM->SBUF copy.
Matmul inputs bf16, fp32 PSUM/softmax.
"""
from collections import deque

import numpy as np
import ml_dtypes

import concourse.bass as bass
import concourse.mybir as mybir
import concourse.tile as tile
from concourse import bacc
from concourse.bass_utils import run_bass_kernel_spmd

B, S, D, H, HD = 2, 4096, 768, 12, 64
W = 256
G = 32
SCALE = 1.0 / np.float32(np.sqrt(HD))
KB = 128
NKB = S // KB     # 32
QSB = 512
NQSB = S // QSB   # 8
NKT = D // 128    # 6
NNT = S // 512    # 8

BF = mybir.dt.bfloat16
F32 = mybir.dt.float32
AF = mybir.ActivationFunctionType
bf16 = ml_dtypes.bfloat16

_cache = {}
PUMP_INTERLEAVE = True


def _span(kb):
    k0 = KB * kb
    qlo, qhi = max(0, k0 - 2 * KB), min(S, k0 + 3 * KB)
    return qlo, qhi, qlo - (k0 - 2 * KB), qhi - (k0 - 2 * KB)


def _build():
    nc = bacc.Bacc(None, target_bir_lowering=False)

    hsT_d = nc.declare_dram_parameter("hsT", [128, NNT, NKT, 512], BF, isOutput=False)
    w5_d = nc.declare_dram_parameter("w5", [128, 5, NKT, 128], BF, isOutput=False)
    wqg_d = nc.declare_dram_parameter("wqg01", [128, NKT, 128], BF, isOutput=False)
    wvvg_d = nc.declare_dram_parameter("wvvg", [128, NKT, 384], BF, isOutput=False)
    bvvg_d = nc.declare_dram_parameter("bvvg", [1, 384], BF, isOutput=False)
    bias_d = nc.declare_dram_parameter("bias_t", [128, 8], F32, isOutput=False)
    masks_d = nc.declare_dram_parameter("masks", [128, 2, 128], BF, isOutput=False)
    id96_d = nc.declare_dram_parameter("id96", [96, 96], BF, isOutput=False)
    out_d = nc.declare_dram_parameter("out", [3, 65, S], F32, isOutput=True)
    outg_d = nc.declare_dram_parameter("outg", [3, G, 65], F32, isOutput=True)

    with tile.TileContext(nc) as tc:
        with tc.tile_pool(name="persist", bufs=1) as pp:
            masks_t = pp.tile([128, 2, 128], BF)
            ones_t = pp.tile([1, 128], BF)

            # per-head zero-padded q; head h's live rows match its k tile rows
            qZ = [pp.tile([128, S], BF, name=f"qZ{i}") for i in range(3)]
            kT01 = pp.tile([128, S], BF)   # k: h0 rows 0:64, h1 rows 64:128
            kT2 = pp.tile([128, S], BF)    # k: h2 rows 0:64, rows 64:128 zero
            v_nat = pp.tile([128, NKB, 3, 65], BF)
            # exp_sg: rows 32h..32h+31 = head h's exp'd global-key scores;
            # rows 96:128 stay zero so K=128 matmuls vs vGp are exact.
            exp_sg = pp.tile([128, S], BF)

            with tc.tile_pool(name="ac", bufs=1) as ac:
                kgT01 = ac.tile([128, S], BF)
                kgT2 = ac.tile([128, S], BF)   # h2 rows 64:128, rows 0:64 zero
                qgZ = [ac.tile([128, G], BF, name=f"qgZ{i}") for i in range(3)]
                vg_nat = ac.tile([128, NKB, 3, 65], BF)
                vGp = ac.tile([128, 3, 65], BF)   # head h global-v at rows 32h:32h+32
                probs_g = ac.tile([96, S], BF)
                pb_gT = ac.tile([128, NKB, 96], BF)
                id96_t = ac.tile([96, 96], BF)
                bvvg_b = ac.tile([128, 384], BF)  # bias broadcast over tokens

                with (
                    tc.tile_pool(name="aw", bufs=1) as aw,
                    tc.tile_pool(name="hst", bufs=2) as hstp,
                    tc.tile_pool(name="apsum", bufs=2, space="PSUM") as apsum,
                    tc.tile_pool(name="spsum", bufs=2, space="PSUM") as spsum,
                    tc.tile_pool(name="opsum", bufs=2, space="PSUM") as opsum,
                    tc.tile_pool(name="pbt", bufs=42) as pbtp,
                    tc.tile_pool(name="osb", bufs=2) as osbp,
                ):
                    w5_t = aw.tile([128, 5, NKT, 128], BF)
                    wqg_t = aw.tile([128, NKT, 128], BF)
                    wvvg_t = aw.tile([128, NKT, 384], BF)
                    bvvg_t = aw.tile([1, 384], BF)
                    bias_t = aw.tile([128, 8], F32)
                    # issue startup DMAs sliced across the three DMA-issuing
                    # engines so the critical-path transfers run in parallel
                    hst0 = hstp.tile([128, NKT, 512], BF)
                    nc.sync.dma_start(w5_t[:, 0], w5_d[:, 0])
                    nc.scalar.dma_start(bias_t[:], bias_d[:])
                    nc.gpsimd.dma_start(hst0[:, 2:4, :], hsT_d[:, 0, 2:4, :])
                    nc.scalar.dma_start(hst0[:, 0:2, :], hsT_d[:, 0, 0:2, :])
                    nc.sync.dma_start(hst0[:, 4:6, :], hsT_d[:, 0, 4:6, :])
                    nc.scalar.dma_start(w5_t[:, 1], w5_d[:, 1])
                    nc.gpsimd.dma_start(w5_t[:, 2], w5_d[:, 2])
                    nc.sync.dma_start(w5_t[:, 3], w5_d[:, 3])
                    nc.scalar.dma_start(w5_t[:, 4], w5_d[:, 4])
                    nc.gpsimd.dma_start(wvvg_t[:], wvvg_d[:])
                    nc.sync.dma_start(bvvg_t[:], bvvg_d[:])
                    nc.scalar.dma_start(masks_t[:], masks_d[:])
                    nc.gpsimd.dma_start(wqg_t[:], wqg_d[:])
                    nc.gpsimd.dma_start(id96_t[:], id96_d[:])
                    # zero-fill the dead halves of the padded tiles + ones cols
                    nc.gpsimd.memset(ones_t[:], 1.0)
                    nc.gpsimd.memset(qZ[0][64:128, :], 0.0)
                    nc.gpsimd.memset(qZ[1][0:64, :], 0.0)
                    nc.gpsimd.memset(qZ[2][64:128, :], 0.0)
                    nc.gpsimd.memset(kT2[64:128, :], 0.0)
                    nc.gpsimd.memset(kgT2[0:64, :], 0.0)
                    nc.gpsimd.memset(qgZ[0][64:128, :], 0.0)
                    nc.gpsimd.memset(qgZ[1][0:64, :], 0.0)
                    nc.gpsimd.memset(qgZ[2][0:64, :], 0.0)
                    nc.gpsimd.memset(exp_sg[96:128, :], 0.0)
                    nc.gpsimd.memset(v_nat[:, :, :, 64:65], 1.0)
                    nc.gpsimd.memset(vg_nat[:, :, :, 64:65], 1.0)
                    nc.gpsimd.memset(vGp[:], 0.0)
                    for h in range(3):
                        nc.gpsimd.memset(vGp[32 * h:32 * h + 32, h, 64:65], 1.0)

                    pbt = {}
                    band_q = deque()
                    state = {"kb_done": 0, "qs_done": 0, "kb_next": 0}

                    def ktile(h):
                        return kT01 if h < 2 else kT2

                    def mm_score(t, h, kb, a, b2):
                        k0 = KB * kb
                        qlo, qhi, llo, lhi = _span(kb)
                        nc.tensor.matmul(
                            t[:, a:b2],
                            ktile(h)[:, k0:k0 + KB],
                            qZ[h][:, qlo + (a - llo):qlo + (a - llo) + (b2 - a)])

                    def exp_mask(ps, kb, h):
                        qlo, qhi, llo, lhi = _span(kb)
                        t_ = pbtp.tile([128, 640], BF, tag="pb")
                        nc.scalar.activation(t_[:, llo:lhi], ps[:, llo:lhi], AF.Exp)
                        tv = t_.rearrange("p (o j) -> p o j", o=5)
                        if llo == 0 and lhi == 640:
                            nc.vector.tensor_mul(tv[:, 0:5:4, :], tv[:, 0:5:4, :],
                                                 masks_t[:])
                        elif llo == 0:
                            nc.vector.tensor_mul(tv[:, 0, :], tv[:, 0, :],
                                                 masks_t[:, 0, :])
                        else:
                            nc.vector.tensor_mul(tv[:, 4, :], tv[:, 4, :],
                                                 masks_t[:, 1, :])
                        pbt[(kb, h)] = t_

                    def queue_kb(kb):
                        qlo, qhi, llo, lhi = _span(kb)
                        pieces = [(a, b) for (a, b) in
                                  [(llo, min(lhi, 512)), (max(llo, 512), lhi)] if a < b]

                        def part1():
                            t0 = spsum.tile([128, 640], F32, tag="sc", name=f"s{kb}_0")
                            t1 = spsum.tile([128, 640], F32, tag="sc", name=f"s{kb}_1")
                            for (a, b2) in pieces:
                                mm_score(t0, 0, kb, a, b2)
                                mm_score(t1, 1, kb, a, b2)
                            exp_mask(t0, kb, 0)
                            exp_mask(t1, kb, 1)

                        def part2():
                            t2 = spsum.tile([128, 640], F32, tag="sc", name=f"s{kb}_2")
                            for (a, b2) in pieces:
                                mm_score(t2, 2, kb, a, b2)
                            exp_mask(t2, kb, 2)
                            state["kb_done"] += 1

                        band_q.append(part1)
                        band_q.append(part2)

                    def pump(n=1):
                        if PUMP_INTERLEAVE:
                            for _ in range(n):
                                if band_q:
                                    band_q.popleft()()

                    def pump_all():
                        while band_q:
                            band_q.popleft()()

                    def do_pv(qs):
                        q0 = QSB * qs
                        kbs = list(range(max(0, 4 * qs - 2), min(NKB, 4 * qs + 6)))
                        for h in range(3):
                            po = opsum.tile([96, 512], F32, tag="po")
                            nc.tensor.matmul(po[0:65, :], vGp[:, h, :],
                                             exp_sg[:, q0:q0 + 512],
                                             start=True, stop=False)
                            for i, kb in enumerate(kbs):
                                k0 = KB * kb
                                qlo, qhi, llo, lhi = _span(kb)
                                a, b2 = max(qlo, q0), min(qhi, q0 + QSB)
                                la = a - (k0 - 2 * KB)
                                nc.tensor.matmul(po[0:65, a - q0:b2 - q0],
                                                 v_nat[:, kb, h, :],
                                                 pbt[(kb, h)][:, la:la + (b2 - a)],
                                                 start=False, stop=(i == len(kbs) - 1))
                            ob = osbp.tile([65, 512], F32, tag="ob")
                            nc.vector.tensor_copy(ob[:], po[0:65, :])
                            nc.sync.dma_start(out_d[h, :, q0:q0 + 512], ob[:])
                            pump()

                    def pump_pv(nt):
                        while state["qs_done"] < NQSB \
                                and 4 * state["qs_done"] + 5 <= state["kb_done"] - 1 \
                                and state["qs_done"] <= nt - 1:
                            do_pv(state["qs_done"])
                            state["qs_done"] += 1

                    def do_tpose(blk):
                        pst = apsum.tile([128, 96], BF, tag="pp", name=f"tp{blk}")
                        nc.tensor.transpose(pst[:], probs_g[:, 128 * blk:128 * blk + 128],
                                            id96_t[:])
                        nc.vector.tensor_copy(pb_gT[:, blk, :], pst[:])

                    for nt in range(NNT):
                        c0 = 512 * nt
                        if nt == 0:
                            hst = hst0
                        else:
                            hst = hstp.tile([128, NKT, 512], BF)
                            nc.sync.dma_start(hst[:], hsT_d[:, nt, :, :])
                        # queue band blocks whose q/k spans are fully written
                        # (chunks <= nt-1); the rest after this chunk's m-chains
                        while state["kb_next"] <= min(4 * nt - 3, NKB - 1):
                            queue_kb(state["kb_next"])
                            state["kb_next"] += 1
                        for c in range(5):
                            ps = apsum.tile([128, 512], F32, tag="pp")
                            for kt in range(NKT):
                                nc.tensor.matmul(ps[:], w5_t[:, c, kt, :],
                                                 hst[:, kt, :],
                                                 start=(kt == 0), stop=(kt == NKT - 1))
                            if c == 0:
                                nc.vector.tensor_scalar_add(
                                    qZ[0][0:64, c0:c0 + 512], ps[0:64, :],
                                    bias_t[0:64, 0:1])
                                nc.vector.tensor_scalar_add(
                                    qZ[1][64:128, c0:c0 + 512], ps[64:128, :],
                                    bias_t[64:128, 0:1])
                            elif c == 1:
                                nc.vector.tensor_scalar_add(
                                    kT01[:, c0:c0 + 512], ps[:], bias_t[:, 1:2])
                            elif c == 2:
                                nc.vector.tensor_scalar_add(
                                    kgT01[:, c0:c0 + 512], ps[:], bias_t[:, 2:3])
                            elif c == 3:
                                nc.vector.tensor_scalar_add(
                                    qZ[2][0:64, c0:c0 + 512], ps[0:64, :],
                                    bias_t[0:64, 3:4])
                                nc.vector.tensor_scalar_add(
                                    kgT2[64:128, c0:c0 + 512], ps[64:128, :],
                                    bias_t[64:128, 3:4])
                            else:
                                nc.vector.tensor_scalar_add(
                                    kT2[0:64, c0:c0 + 512], ps[0:64, :],
                                    bias_t[0:64, 4:5])
                                if nt == 0:
                                    nc.vector.tensor_scalar_add(
                                        qgZ[2][64:128, :], ps[64:128, 0:G],
                                        bias_t[64:128, 4:5])
                            pump()
                        while state["kb_next"] <= min(4 * nt + 1, NKB - 1):
                            queue_kb(state["kb_next"])
                            state["kb_next"] += 1
                        if nt == 0:
                            psq = apsum.tile([128, 512], F32, tag="pp")
                            for kt in range(NKT):
                                nc.tensor.matmul(psq[:, 0:G], wqg_t[:, kt, :],
                                                 hst[:, kt, 0:G],
                                                 start=(kt == 0), stop=(kt == NKT - 1))
                            nc.vector.tensor_scalar_add(qgZ[0][0:64, :], psq[0:64, 0:G],
                                                        bias_t[0:64, 5:6])
                            nc.vector.tensor_scalar_add(qgZ[1][64:128, :],
                                                        psq[64:128, 0:G],
                                                        bias_t[64:128, 5:6])
                            # broadcast bvvg to all partitions via ones-matmul
                            psb = apsum.tile([128, 512], F32, tag="pp")
                            nc.tensor.matmul(psb[:, 0:384], ones_t[:, 0:128], bvvg_t[:],
                                             start=True, stop=True)
                            nc.vector.tensor_copy(bvvg_b[:], psb[:, 0:384])
                        for s4 in range(4):
                            sb = 4 * nt + s4
                            psv = apsum.tile([128, 512], F32, tag="pp")
                            for kt in range(NKT):
                                nc.tensor.matmul(psv[:, 0:384],
                                                 hst[:, kt, 128 * s4:128 * s4 + 128],
                                                 wvvg_t[:, kt, :],
                                                 start=(kt == 0), stop=(kt == NKT - 1))
                            nc.vector.tensor_add(
                                v_nat[:, sb, :, 0:64],
                                psv[:, 0:192].rearrange("p (h e) -> p h e", h=3),
                                bvvg_b[:, 0:192].rearrange("p (h e) -> p h e", h=3))
                            nc.vector.tensor_add(
                                vg_nat[:, sb, :, 0:64],
                                psv[:, 192:384].rearrange("p (h e) -> p h e", h=3),
                                bvvg_b[:, 192:384].rearrange("p (h e) -> p h e", h=3))
                            pump()
                        if nt == 0:
                            # per-head global v/ones at partitions 32h:32h+32
                            nc.vector.tensor_copy(vGp[0:32, 0, 0:64],
                                                  v_nat[0:32, 0, 0, 0:64])
                            nc.sync.dma_start(vGp[32:64, 1, 0:64],
                                              v_nat[0:32, 0, 1, 0:64])
                            nc.sync.dma_start(vGp[64:96, 2, 0:64],
                                              v_nat[0:32, 0, 2, 0:64])
                        if nt >= 1:
                            for blk in range(4 * (nt - 1), 4 * nt):
                                do_tpose(blk)
                                pump()
                        pump_pv(nt)
                        pump(2)
                        # ---- (128,32)-col-tiled scores: sg + global-query ----
                        pssg = opsum.tile([96, 512], F32, tag="po", name=f"sg{nt}")
                        for h in range(3):
                            nc.tensor.matmul(pssg[32 * h:32 * h + 32, :],
                                             ktile(h)[:, 0:G],
                                             qZ[h][:, c0:c0 + 512])
                        nc.scalar.activation(exp_sg[0:96, c0:c0 + 512], pssg[:], AF.Exp)
                        psgc = opsum.tile([96, 512], F32, tag="po", name=f"gc{nt}")
                        for h in range(3):
                            nc.tensor.matmul(psgc[32 * h:32 * h + 32, :],
                                             qgZ[h][:],
                                             (kgT01 if h < 2 else kgT2)[:, c0:c0 + 512])
                        nc.scalar.activation(probs_g[:, c0:c0 + 512], psgc[:], AF.Exp)
                        pump(2)

                    # tail: drain band queue, pv(6); the global-query PV chain
                    # runs before pv(7) so its copies/DMA overlap pv(7)'s matmuls
                    pump_all()
                    while state["qs_done"] <= 6:
                        do_pv(state["qs_done"])
                        state["qs_done"] += 1
                    for blk in range(4 * (NNT - 1), NKB):
                        do_tpose(blk)
                    while state["kb_next"] < NKB:
                        queue_kb(state["kb_next"])
                        state["kb_next"] += 1
                    pump_all()
                    do_pv(7)
                    go = apsum.tile([128, 512], F32, tag="pp", name="go")
                    og_sb = osbp.tile([96, 65], F32, tag="og")
                    for t in range(NKB):
                        for h in range(3):
                            nc.tensor.matmul(go[32 * h:32 * h + 32, 0:65],
                                             pb_gT[:, t, 32 * h:32 * h + 32],
                                             vg_nat[:, t, h, :],
                                             start=(t == 0), stop=(t == NKB - 1))
                    nc.vector.tensor_copy(og_sb[:], go[0:96, 0:65])
                    nc.sync.dma_start(outg_d[0], og_sb[0:32, :])
                    nc.scalar.dma_start(outg_d[1], og_sb[32:64, :])
                    nc.gpsimd.dma_start(outg_d[2], og_sb[64:96, :])

    nc.compile()
    return nc


def _prep_inputs(inputs):
    hs = np.asarray(inputs["hidden_states"], dtype=np.float32)
    j = np.arange(KB)[None, :]
    p = np.arange(KB)[:, None]
    masks = np.stack([(j >= p), (j <= p)], axis=1).astype(bf16)  # [128, 2, 128]
    id96 = np.eye(96, dtype=bf16)

    def wtiles(w):
        n = w.shape[1]
        return np.ascontiguousarray(w.reshape(NKT, 128, n).transpose(1, 0, 2)).astype(bf16)

    maps = []
    for c in range(8):
        b, hg = c // 4, c % 4
        cols = slice(192 * hg, 192 * hg + 192)
        Wq = np.asarray(inputs["Wq"], np.float32)[:, cols] * SCALE
        bq = np.asarray(inputs["bq"], np.float32)[cols] * SCALE
        Wqg = np.asarray(inputs["Wqg"], np.float32)[:, cols] * SCALE
        bqg = np.asarray(inputs["bqg"], np.float32)[cols] * SCALE
        Wk = np.asarray(inputs["Wk"], np.float32)[:, cols]
        bk = np.asarray(inputs["bk"], np.float32)[cols]
        Wkg = np.asarray(inputs["Wkg"], np.float32)[:, cols]
        bkg = np.asarray(inputs["bkg"], np.float32)[cols]
        Wv = np.asarray(inputs["Wv"], np.float32)[:, cols]
        bv = np.asarray(inputs["bv"], np.float32)[cols]
        Wvg = np.asarray(inputs["Wvg"], np.float32)[:, cols]
        bvg = np.asarray(inputs["bvg"], np.float32)[cols]

        # 5 chains: q01, k01, kg01, [q2|kg2], [k2|qg2]
        w5 = np.concatenate([
            Wq[:, 0:128],
            Wk[:, 0:128],
            Wkg[:, 0:128],
            np.concatenate([Wq[:, 128:192], Wkg[:, 128:192]], axis=1),
            np.concatenate([Wk[:, 128:192], Wqg[:, 128:192]], axis=1),
        ], axis=1)  # [768, 640]
        w5_t = np.ascontiguousarray(
            wtiles(w5).reshape(128, NKT, 5, 128).transpose(0, 2, 1, 3))

        bias_t = np.zeros((128, 8), np.float32)
        bias_t[:, 0] = bq[0:128]
        bias_t[:, 1] = bk[0:128]
        bias_t[:, 2] = bkg[0:128]
        bias_t[0:64, 3], bias_t[64:128, 3] = bq[128:192], bkg[128:192]
        bias_t[0:64, 4], bias_t[64:128, 4] = bk[128:192], bqg[128:192]
        bias_t[0:64, 5], bias_t[64:128, 5] = bqg[0:64], bqg[64:128]

        hsT = np.ascontiguousarray(
            hs[b].T.reshape(NKT, 128, NNT, 512).transpose(1, 2, 0, 3)).astype(bf16)

        maps.append({
            "hsT": hsT,
            "w5": w5_t,
            "wqg01": wtiles(Wqg[:, 0:128]),
            "wvvg": wtiles(np.concatenate([Wv, Wvg], axis=1)),
            "bvvg": np.concatenate([bv, bvg])[None, :].astype(bf16),
            "bias_t": bias_t,
            "masks": masks,
            "id96": id96,
        })
    return maps


def kernel(**inputs):
    g = int(np.asarray(inputs["num_global"]))
    assert g == G, f"kernel compiled for num_global=32, got {g}"
    if "nc" not in _cache:
        _cache["nc"] = _build()
    nc = _cache["nc"]
    in_maps = _prep_inputs(inputs)
    res = run_bass_kernel_spmd(nc, in_maps, list(range(8)))
    return assemble(res.results)


def assemble(results):
    out = np.zeros((B, S, D), np.float32)
    for c in range(8):
        b, hg = c // 4, c % 4
        o = results[c]["out"]          # [3, 65, S]
        og = results[c]["outg"]        # [3, G, 65]
        for h in range(3):
            col = 192 * hg + 64 * h
            out[b, :, col:col + 64] = (o[h, 0:64] / o[h, 64]).T
            out[b, 0:G, col:col + 64] = og[h, :, 0:64] / og[h, :, 64:65]
    return out


# revision 39
# speedup vs baseline: 1.0494x; 1.0012x over previous
"""Trainium2 Bass kernel for Longformer self-attention (B=2, S=4096, D=768, H=12, HD=64, W=256, G=32).

Sharding: 8 cores = 2 batches x 4 head-groups (3 heads each). Each core computes its
batch's projections restricted to its 192 output channels, runs banded + global
attention for its 3 heads, and returns an unnormalized transposed output
([3, 65, S]: rows 0-63 = head-dim, row 64 = softmax denominator z) plus the raw
global-query output [3, G, 65]; the host divides by z, transposes, and assembles.

Key design (v4):
  - All score matmuls run at K=128 full-array rate via ZERO-PADDED per-head
    query tiles (qZ[h] holds head h's 64 q-channels on the partition range of
    its k data, zeros elsewhere; the packed k tiles' cross-head terms are
    killed by the zeros). K=64 matmuls with changing weights cannot pipeline
    on the PE (measured fill+drain serialization, ~2x cost) -- avoided.
  - hidden_states pre-transposed on host -> contiguous DMA loads; startup DMAs
    sliced across the three DMA-issuing engines (sync/scalar/gpsimd).
  - 5 projection chains of full 128 output rows (q01, k01, kg01, [q2|kg2],
    [k2|qg2]).
  - band score blocks interleaved between projection/PV chains (same PE mode)
    so PSUM-slot waits on the softmax exp hide behind useful matmuls.
  - band mask applied as one strided bf16 multiply on the exp'd probs.
  - global-key PV contribution stays K=128 via zero-padded operands
    (exp_sg rows 96:128 = 0, per-head global-v at partitions 32h:32h+32).
  - v/vg bias via pre-broadcast bf16 add fused into the PSUM->SBUF copy.
Matmul inputs bf16, fp32 PSUM/softmax.
"""
from collections import deque

import numpy as np
import ml_dtypes

import concourse.bass as bass
import concourse.mybir as mybir
import concourse.tile as tile
from concourse import bacc
from concourse.bass_utils import run_bass_kernel_spmd

B, S, D, H, HD = 2, 4096, 768, 12, 64
W = 256
G = 32
SCALE = 1.0 / np.float32(np.sqrt(HD))
KB = 128
NKB = S // KB     # 32
QSB = 512
NQSB = S // QSB   # 8
NKT = D // 128    # 6
NNT = S // 512    # 8

BF = mybir.dt.bfloat16
F32 = mybir.dt.float32
AF = mybir.ActivationFunctionType
bf16 = ml_dtypes.bfloat16

_cache = {}
PUMP_INTERLEAVE = True


def _span(kb):
    k0 = KB * kb
    qlo, qhi = max(0, k0 - 2 * KB), min(S, k0 + 3 * KB)
    return qlo, qhi, qlo - (k0 - 2 * KB), qhi - (k0 - 2 * KB)


def _build():
    nc = bacc.Bacc(None, target_bir_lowering=False)

    hsT_d = nc.declare_dram_parameter("hsT", [128, NNT, NKT, 512], BF, isOutput=False)
    w5_d = nc.declare_dram_parameter("w5", [128, 5, NKT, 128], BF, isOutput=False)
    wqg_d = nc.declare_dram_parameter("wqg01", [128, NKT, 128], BF, isOutput=False)
    wvvg_d = nc.declare_dram_parameter("wvvg", [128, NKT, 384], BF, isOutput=False)
    bvvg_d = nc.declare_dram_parameter("bvvg", [1, 384], BF, isOutput=False)
    bias_d = nc.declare_dram_parameter("bias_t", [128, 8], F32, isOutput=False)
    masks_d = nc.declare_dram_parameter("masks", [128, 2, 128], BF, isOutput=False)
    id96_d = nc.declare_dram_parameter("id96", [96, 96], BF, isOutput=False)
    out_d = nc.declare_dram_parameter("out", [3, 65, S], F32, isOutput=True)
    outg_d = nc.declare_dram_parameter("outg", [3, G, 65], F32, isOutput=True)

    with tile.TileContext(nc) as tc:
        with tc.tile_pool(name="persist", bufs=1) as pp:
            masks_t = pp.tile([128, 2, 128], BF)
            ones_t = pp.tile([1, 128], BF)

            # per-head zero-padded q; head h's live rows match its k tile rows
            qZ = [pp.tile([128, S], BF, name=f"qZ{i}") for i in range(3)]
            kT01 = pp.tile([128, S], BF)   # k: h0 rows 0:64, h1 rows 64:128
            kT2 = pp.tile([128, S], BF)    # k: h2 rows 0:64, rows 64:128 zero
            v_nat = pp.tile([128, NKB, 6, 65], BF)  # idx 0:3 = v, 3:6 = vg
            # exp_sg: rows 32h..32h+31 = head h's exp'd global-key scores;
            # rows 96:128 stay zero so K=128 matmuls vs vGp are exact.
            exp_sg = pp.tile([128, S], BF)

            with tc.tile_pool(name="ac", bufs=1) as ac:
                kgT01 = ac.tile([128, S], BF)
                kgT2 = ac.tile([128, S], BF)   # h2 rows 64:128, rows 0:64 zero
                qgZ = [ac.tile([128, G], BF, name=f"qgZ{i}") for i in range(3)]
                vGp = ac.tile([128, 3, 65], BF)   # head h global-v at rows 32h:32h+32
                probs_g = ac.tile([96, S], BF)
                pb_gT = ac.tile([128, NKB, 96], BF)
                id96_t = ac.tile([96, 96], BF)
                bvvg_b = ac.tile([128, 384], BF)  # bias broadcast over tokens

                with (
                    tc.tile_pool(name="aw", bufs=1) as aw,
                    tc.tile_pool(name="hst", bufs=2) as hstp,
                    tc.tile_pool(name="apsum", bufs=2, space="PSUM") as apsum,
                    tc.tile_pool(name="spsum", bufs=2, space="PSUM") as spsum,
                    tc.tile_pool(name="opsum", bufs=2, space="PSUM") as opsum,
                    tc.tile_pool(name="pbt", bufs=42) as pbtp,
                    tc.tile_pool(name="osb", bufs=2) as osbp,
                ):
                    w5_t = aw.tile([128, 5, NKT, 128], BF)
                    wqg_t = aw.tile([128, NKT, 128], BF)
                    wvvg_t = aw.tile([128, NKT, 384], BF)
                    bvvg_t = aw.tile([1, 384], BF)
                    bias_t = aw.tile([128, 8], F32)
                    # issue startup DMAs sliced across the three DMA-issuing
                    # engines so the critical-path transfers run in parallel
                    hst0 = hstp.tile([128, NKT, 512], BF)
                    nc.sync.dma_start(w5_t[:, 0, 0:3], w5_d[:, 0, 0:3])
                    nc.scalar.dma_start(bias_t[:], bias_d[:])
                    nc.gpsimd.dma_start(hst0[:, 0:1, :], hsT_d[:, 0, 0:1, :])
                    nc.scalar.dma_start(hst0[:, 1:3, :], hsT_d[:, 0, 1:3, :])
                    nc.sync.dma_start(w5_t[:, 0, 3:6], w5_d[:, 0, 3:6])
                    nc.gpsimd.dma_start(hst0[:, 3:6, :], hsT_d[:, 0, 3:6, :])
                    nc.scalar.dma_start(w5_t[:, 1], w5_d[:, 1])
                    nc.gpsimd.dma_start(w5_t[:, 2], w5_d[:, 2])
                    nc.sync.dma_start(w5_t[:, 3], w5_d[:, 3])
                    nc.scalar.dma_start(w5_t[:, 4], w5_d[:, 4])
                    nc.gpsimd.dma_start(wvvg_t[:], wvvg_d[:])
                    nc.sync.dma_start(bvvg_t[:], bvvg_d[:])
                    nc.scalar.dma_start(masks_t[:], masks_d[:])
                    nc.gpsimd.dma_start(wqg_t[:], wqg_d[:])
                    nc.gpsimd.dma_start(id96_t[:], id96_d[:])
                    # zero-fill the dead halves of the padded tiles + ones cols
                    nc.gpsimd.memset(ones_t[:], 1.0)
                    nc.gpsimd.memset(qZ[0][64:128, :], 0.0)
                    nc.gpsimd.memset(qZ[1][0:64, :], 0.0)
                    nc.gpsimd.memset(qZ[2][64:128, :], 0.0)
                    nc.gpsimd.memset(kT2[64:128, :], 0.0)
                    nc.gpsimd.memset(kgT2[0:64, :], 0.0)
                    nc.gpsimd.memset(qgZ[0][64:128, :], 0.0)
                    nc.gpsimd.memset(qgZ[1][0:64, :], 0.0)
                    nc.gpsimd.memset(qgZ[2][0:64, :], 0.0)
                    nc.gpsimd.memset(exp_sg[96:128, :], 0.0)
                    nc.gpsimd.memset(v_nat[:, :, :, 64:65], 1.0)
                    nc.gpsimd.memset(vGp[:], 0.0)
                    for h in range(3):
                        nc.gpsimd.memset(vGp[32 * h:32 * h + 32, h, 64:65], 1.0)

                    pbt = {}
                    band_q = deque()
                    state = {"kb_done": 0, "qs_done": 0, "kb_next": 0}

                    def ktile(h):
                        return kT01 if h < 2 else kT2

                    def mm_score(t, h, kb, a, b2):
                        k0 = KB * kb
                        qlo, qhi, llo, lhi = _span(kb)
                        nc.tensor.matmul(
                            t[:, a:b2],
                            ktile(h)[:, k0:k0 + KB],
                            qZ[h][:, qlo + (a - llo):qlo + (a - llo) + (b2 - a)])

                    def exp_mask(ps, kb, h):
                        qlo, qhi, llo, lhi = _span(kb)
                        t_ = pbtp.tile([128, 640], BF, tag="pb")
                        nc.scalar.activation(t_[:, llo:lhi], ps[:, llo:lhi], AF.Exp)
                        tv = t_.rearrange("p (o j) -> p o j", o=5)
                        if llo == 0 and lhi == 640:
                            nc.vector.tensor_mul(tv[:, 0:5:4, :], tv[:, 0:5:4, :],
                                                 masks_t[:])
                        elif llo == 0:
                            nc.vector.tensor_mul(tv[:, 0, :], tv[:, 0, :],
                                                 masks_t[:, 0, :])
                        else:
                            nc.vector.tensor_mul(tv[:, 4, :], tv[:, 4, :],
                                                 masks_t[:, 1, :])
                        pbt[(kb, h)] = t_

                    def queue_kb(kb):
                        qlo, qhi, llo, lhi = _span(kb)
                        pieces = [(a, b) for (a, b) in
                                  [(llo, min(lhi, 512)), (max(llo, 512), lhi)] if a < b]

                        def part1():
                            t0 = spsum.tile([128, 640], F32, tag="sc", name=f"s{kb}_0")
                            t1 = spsum.tile([128, 640], F32, tag="sc", name=f"s{kb}_1")
                            for (a, b2) in pieces:
                                mm_score(t0, 0, kb, a, b2)
                                mm_score(t1, 1, kb, a, b2)
                            exp_mask(t0, kb, 0)
                            exp_mask(t1, kb, 1)

                        def part2():
                            t2 = spsum.tile([128, 640], F32, tag="sc", name=f"s{kb}_2")
                            for (a, b2) in pieces:
                                mm_score(t2, 2, kb, a, b2)
                            exp_mask(t2, kb, 2)
                            state["kb_done"] += 1

                        band_q.append(part1)
                        band_q.append(part2)

                    def pump(n=1):
                        if PUMP_INTERLEAVE:
                            for _ in range(n):
                                if band_q:
                                    band_q.popleft()()

                    def pump_all():
                        while band_q:
                            band_q.popleft()()

                    def do_pv(qs):
                        q0 = QSB * qs
                        kbs = list(range(max(0, 4 * qs - 2), min(NKB, 4 * qs + 6)))
                        for h in range(3):
                            po = opsum.tile([96, 512], F32, tag="po")
                            nc.tensor.matmul(po[0:65, :], vGp[:, h, :],
                                             exp_sg[:, q0:q0 + 512],
                                             start=True, stop=False)
                            for i, kb in enumerate(kbs):
                                k0 = KB * kb
                                qlo, qhi, llo, lhi = _span(kb)
                                a, b2 = max(qlo, q0), min(qhi, q0 + QSB)
                                la = a - (k0 - 2 * KB)
                                nc.tensor.matmul(po[0:65, a - q0:b2 - q0],
                                                 v_nat[:, kb, h, :],
                                                 pbt[(kb, h)][:, la:la + (b2 - a)],
                                                 start=False, stop=(i == len(kbs) - 1))
                            ob = osbp.tile([65, 512], F32, tag="ob")
                            if h == 1:
                                nc.scalar.copy(ob[:], po[0:65, :])
                            else:
                                nc.vector.tensor_copy(ob[:], po[0:65, :])
                            nc.sync.dma_start(out_d[h, :, q0:q0 + 512], ob[:])
                            pump()

                    def pump_pv(nt):
                        while state["qs_done"] < NQSB \
                                and 4 * state["qs_done"] + 5 <= state["kb_done"] - 1 \
                                and state["qs_done"] <= nt - 1:
                            do_pv(state["qs_done"])
                            state["qs_done"] += 1

                    def do_tpose(blk):
                        pst = apsum.tile([128, 96], BF, tag="pp", name=f"tp{blk}")
                        nc.tensor.transpose(pst[:], probs_g[:, 128 * blk:128 * blk + 128],
                                            id96_t[:])
                        nc.vector.tensor_copy(pb_gT[:, blk, :], pst[:])

                    for nt in range(NNT):
                        c0 = 512 * nt
                        if nt == 0:
                            hst = hst0
                        else:
                            hst = hstp.tile([128, NKT, 512], BF)
                            nc.sync.dma_start(hst[:], hsT_d[:, nt, :, :])
                        # queue band blocks whose q/k spans are fully written
                        # (chunks <= nt-1); the rest after this chunk's m-chains
                        while state["kb_next"] <= min(4 * nt - 3, NKB - 1):
                            queue_kb(state["kb_next"])
                            state["kb_next"] += 1
                        for c in range(5):
                            ps = apsum.tile([128, 512], F32, tag="pp")
                            for kt in range(NKT):
                                nc.tensor.matmul(ps[:], w5_t[:, c, kt, :],
                                                 hst[:, kt, :],
                                                 start=(kt == 0), stop=(kt == NKT - 1))
                            if c == 0:
                                nc.vector.tensor_scalar_add(
                                    qZ[0][0:64, c0:c0 + 512], ps[0:64, :],
                                    bias_t[0:64, 0:1])
                                nc.vector.tensor_scalar_add(
                                    qZ[1][64:128, c0:c0 + 512], ps[64:128, :],
                                    bias_t[64:128, 0:1])
                            elif c == 1:
                                nc.vector.tensor_scalar_add(
                                    kT01[:, c0:c0 + 512], ps[:], bias_t[:, 1:2])
                            elif c == 2:
                                nc.vector.tensor_scalar_add(
                                    kgT01[:, c0:c0 + 512], ps[:], bias_t[:, 2:3])
                            elif c == 3:
                                nc.vector.tensor_scalar_add(
                                    qZ[2][0:64, c0:c0 + 512], ps[0:64, :],
                                    bias_t[0:64, 3:4])
                                nc.vector.tensor_scalar_add(
                                    kgT2[64:128, c0:c0 + 512], ps[64:128, :],
                                    bias_t[64:128, 3:4])
                            else:
                                nc.vector.tensor_scalar_add(
                                    kT2[0:64, c0:c0 + 512], ps[0:64, :],
                                    bias_t[0:64, 4:5])
                                if nt == 0:
                                    nc.vector.tensor_scalar_add(
                                        qgZ[2][64:128, :], ps[64:128, 0:G],
                                        bias_t[64:128, 4:5])
                            pump()
                        while state["kb_next"] <= min(4 * nt + 1, NKB - 1):
                            queue_kb(state["kb_next"])
                            state["kb_next"] += 1
                        if nt == 0:
                            psq = apsum.tile([128, 512], F32, tag="pp")
                            for kt in range(NKT):
                                nc.tensor.matmul(psq[:, 0:G], wqg_t[:, kt, :],
                                                 hst[:, kt, 0:G],
                                                 start=(kt == 0), stop=(kt == NKT - 1))
                            nc.vector.tensor_scalar_add(qgZ[0][0:64, :], psq[0:64, 0:G],
                                                        bias_t[0:64, 5:6])
                            nc.vector.tensor_scalar_add(qgZ[1][64:128, :],
                                                        psq[64:128, 0:G],
                                                        bias_t[64:128, 5:6])
                            # broadcast bvvg to all partitions via ones-matmul
                            psb = apsum.tile([128, 512], F32, tag="pp")
                            nc.tensor.matmul(psb[:, 0:384], ones_t[:, 0:128], bvvg_t[:],
                                             start=True, stop=True)
                            nc.vector.tensor_copy(bvvg_b[:], psb[:, 0:384])
                        for s4 in range(4):
                            sb = 4 * nt + s4
                            psv = apsum.tile([128, 512], F32, tag="pp")
                            for kt in range(NKT):
                                nc.tensor.matmul(psv[:, 0:384],
                                                 hst[:, kt, 128 * s4:128 * s4 + 128],
                                                 wvvg_t[:, kt, :],
                                                 start=(kt == 0), stop=(kt == NKT - 1))
                            nc.vector.tensor_add(
                                v_nat[:, sb, :, 0:64],
                                psv[:, 0:384].rearrange("p (h e) -> p h e", h=6),
                                bvvg_b[:, 0:384].rearrange("p (h e) -> p h e", h=6))
                            pump()
                        if nt == 0:
                            # per-head global v/ones at partitions 32h:32h+32
                            nc.vector.tensor_copy(vGp[0:32, 0, 0:64],
                                                  v_nat[0:32, 0, 0, 0:64])
                            nc.sync.dma_start(vGp[32:64, 1, 0:64],
                                              v_nat[0:32, 0, 1, 0:64])
                            nc.sync.dma_start(vGp[64:96, 2, 0:64],
                                              v_nat[0:32, 0, 2, 0:64])
                        if nt >= 1:
                            for blk in range(4 * (nt - 1), 4 * nt):
                                do_tpose(blk)
                                pump()
                        pump_pv(nt)
                        pump(2)
                        # ---- (128,32)-col-tiled scores: sg + global-query ----
                        pssg = opsum.tile([96, 512], F32, tag="po", name=f"sg{nt}")
                        for h in range(3):
                            nc.tensor.matmul(pssg[32 * h:32 * h + 32, :],
                                             ktile(h)[:, 0:G],
                                             qZ[h][:, c0:c0 + 512])
                        nc.scalar.activation(exp_sg[0:96, c0:c0 + 512], pssg[:], AF.Exp)
                        psgc = opsum.tile([96, 512], F32, tag="po", name=f"gc{nt}")
                        for h in range(3):
                            nc.tensor.matmul(psgc[32 * h:32 * h + 32, :],
                                             qgZ[h][:],
                                             (kgT01 if h < 2 else kgT2)[:, c0:c0 + 512])
                        nc.scalar.activation(probs_g[:, c0:c0 + 512], psgc[:], AF.Exp)
                        pump(2)

                    # tail: drain band queue, pv(6); the global-query PV chain
                    # runs before pv(7) so its copies/DMA overlap pv(7)'s matmuls
                    pump_all()
                    while state["qs_done"] <= 6:
                        do_pv(state["qs_done"])
                        state["qs_done"] += 1
                    for blk in range(4 * (NNT - 1), NKB):
                        do_tpose(blk)
                    while state["kb_next"] < NKB:
                        queue_kb(state["kb_next"])
                        state["kb_next"] += 1
                    pump_all()
                    do_pv(7)
                    go = apsum.tile([128, 512], F32, tag="pp", name="go")
                    og_sb = osbp.tile([96, 65], F32, tag="og")
                    for t in range(NKB):
                        for h in range(3):
                            nc.tensor.matmul(go[32 * h:32 * h + 32, 0:65],
                                             pb_gT[:, t, 32 * h:32 * h + 32],
                                             v_nat[:, t, 3 + h, :],
                                             start=(t == 0), stop=(t == NKB - 1))
                    nc.vector.tensor_copy(og_sb[:], go[0:96, 0:65])
                    nc.sync.dma_start(outg_d[0], og_sb[0:32, :])
                    nc.scalar.dma_start(outg_d[1], og_sb[32:64, :])
                    nc.gpsimd.dma_start(outg_d[2], og_sb[64:96, :])

    nc.compile()
    return nc


def _prep_inputs(inputs):
    hs = np.asarray(inputs["hidden_states"], dtype=np.float32)
    j = np.arange(KB)[None, :]
    p = np.arange(KB)[:, None]
    masks = np.stack([(j >= p), (j <= p)], axis=1).astype(bf16)  # [128, 2, 128]
    id96 = np.eye(96, dtype=bf16)

    def wtiles(w):
        n = w.shape[1]
        return np.ascontiguousarray(w.reshape(NKT, 128, n).transpose(1, 0, 2)).astype(bf16)

    maps = []
    for c in range(8):
        b, hg = c // 4, c % 4
        cols = slice(192 * hg, 192 * hg + 192)
        Wq = np.asarray(inputs["Wq"], np.float32)[:, cols] * SCALE
        bq = np.asarray(inputs["bq"], np.float32)[cols] * SCALE
        Wqg = np.asarray(inputs["Wqg"], np.float32)[:, cols] * SCALE
        bqg = np.asarray(inputs["bqg"], np.float32)[cols] * SCALE
        Wk = np.asarray(inputs["Wk"], np.float32)[:, cols]
        bk = np.asarray(inputs["bk"], np.float32)[cols]
        Wkg = np.asarray(inputs["Wkg"], np.float32)[:, cols]
        bkg = np.asarray(inputs["bkg"], np.float32)[cols]
        Wv = np.asarray(inputs["Wv"], np.float32)[:, cols]
        bv = np.asarray(inputs["bv"], np.float32)[cols]
        Wvg = np.asarray(inputs["Wvg"], np.float32)[:, cols]
        bvg = np.asarray(inputs["bvg"], np.float32)[cols]

        # 5 chains: q01, k01, kg01, [q2|kg2], [k2|qg2]
        w5 = np.concatenate([
            Wq[:, 0:128],
            Wk[:, 0:128],
            Wkg[:, 0:128],
            np.concatenate([Wq[:, 128:192], Wkg[:, 128:192]], axis=1),
            np.concatenate([Wk[:, 128:192], Wqg[:, 128:192]], axis=1),
        ], axis=1)  # [768, 640]
        w5_t = np.ascontiguousarray(
            wtiles(w5).reshape(128, NKT, 5, 128).transpose(0, 2, 1, 3))

        bias_t = np.zeros((128, 8), np.float32)
        bias_t[:, 0] = bq[0:128]
        bias_t[:, 1] = bk[0:128]
        bias_t[:, 2] = bkg[0:128]
        bias_t[0:64, 3], bias_t[64:128, 3] = bq[128:192], bkg[128:192]
        bias_t[0:64, 4], bias_t[64:128, 4] = bk[128:192], bqg[128:192]
        bias_t[0:64, 5], bias_t[64:128, 5] = bqg[0:64], bqg[64:128]

        hsT = np.ascontiguousarray(
            hs[b].T.reshape(NKT, 128, NNT, 512).transpose(1, 2, 0, 3)).astype(bf16)

        maps.append({
            "hsT": hsT,
            "w5": w5_t,
            "wqg01": wtiles(Wqg[:, 0:128]),
            "wvvg": wtiles(np.concatenate([Wv, Wvg], axis=1)),
            "bvvg": np.concatenate([bv, bvg])[None, :].astype(bf16),
            "bias_t": bias_t,
            "masks": masks,
            "id96": id96,
        })
    return maps


def kernel(**inputs):
    g = int(np.asarray(inputs["num_global"]))
    assert g == G, f"kernel compiled for num_global=32, got {g}"
    if "nc" not in _cache:
        _cache["nc"] = _build()
    nc = _cache["nc"]
    in_maps = _prep_inputs(inputs)
    res = run_bass_kernel_spmd(nc, in_maps, list(range(8)))
    return assemble(res.results)


def assemble(results):
    out = np.zeros((B, S, D), np.float32)
    for c in range(8):
        b, hg = c // 4, c % 4
        o = results[c]["out"]          # [3, 65, S]
        og = results[c]["outg"]        # [3, G, 65]
        for h in range(3):
            col = 192 * hg + 64 * h
            out[b, :, col:col + 64] = (o[h, 0:64] / o[h, 64]).T
            out[b, 0:G, col:col + 64] = og[h, :, 0:64] / og[h, :, 64:65]
    return out
